# revision 8
# baseline (speedup 1.0000x reference)
"""GRVQ kernel for trn2: conv1d+pre fused, grouped residual VQ (8 quantizers,
2 groups, 1024 codes), post linear. Batch-parallel over 8 cores.

Distance matmuls use fp16 hi/lo 3-pass decomposition (fp32-grade exactness at
3 cyc/row). Argmin via DVE max8/max_index on negated-metric scores
(s = dots - c2/2, argmax s == argmin dist). Residual update via indirect-DMA
gather of fp32 codebook rows + PE transpose + DVE subtract.
"""
import numpy as np

B, DIM, L = 8, 512, 4096
GROUPS, NQ, CSIZE, DG = 2, 8, 1024, 256
NP16 = NQ * GROUPS        # 16 (q,g) pairs, p = q*2 + g
CHUNK = 1024
NCHUNK = L // CHUNK       # 4
PT = CHUNK // 128         # 8 pos-tiles per chunk
NSLOT = 2 * NCHUNK        # loss slots per (q,g): kt x chunk


def _build():
    import concourse.bacc as bacc
    import concourse.mybir as mybir
    from concourse.tile import TileContext
    from concourse.masks import make_identity
    import concourse.bass as bass

    dt = mybir.dt
    nc = bacc.Bacc(None, target_bir_lowering=False, debug=False)

    xp = nc.declare_dram_parameter("xp", [4, 128, L + 2], dt.float32, isOutput=False)
    wf = nc.declare_dram_parameter("wf", [3, 4, 128, DIM], dt.float32, isOutput=False)
    b1 = nc.declare_dram_parameter("b1", [4, 128], dt.float32, isOutput=False)
    cbh = nc.declare_dram_parameter("cbh", [NP16, 2, 128, CSIZE], dt.float16, isOutput=False)
    cbl = nc.declare_dram_parameter("cbl", [NP16, 2, 128, CSIZE], dt.float16, isOutput=False)
    c2hl = nc.declare_dram_parameter("c2hl", [NP16, 2, CSIZE], dt.float16, isOutput=False)
    cbr = nc.declare_dram_parameter("cbr", [NP16 * CSIZE, DG], dt.float32, isOutput=False)
    pwT = nc.declare_dram_parameter("pwT", [4, 128, DIM], dt.float32, isOutput=False)
    pb = nc.declare_dram_parameter("pb", [4, 128], dt.float32, isOutput=False)
    qo = nc.declare_dram_parameter("qo", [DIM, L], dt.float32, isOutput=True)
    io = nc.declare_dram_parameter("io", [GROUPS, NQ, L], dt.uint32, isOutput=True)
    lo = nc.declare_dram_parameter("lo", [1, NP16], dt.float32, isOutput=True)

    AF = mybir.ActivationFunctionType

    with TileContext(nc) as tc:
        with (
            tc.tile_pool(name="const", bufs=1) as cpool,
            tc.tile_pool(name="wpool", bufs=1) as wpool,
            tc.tile_pool(name="xpool", bufs=2) as xpool,
            tc.tile_pool(name="hr", bufs=1) as hrpool,
            tc.tile_pool(name="cb", bufs=2) as cbpool,
            tc.tile_pool(name="hl", bufs=2) as hlpool,
            tc.tile_pool(name="dist", bufs=3) as dpool,
            tc.tile_pool(name="m8", bufs=3) as m8pool,
            tc.tile_pool(name="idx", bufs=2) as ipool,
            tc.tile_pool(name="qg", bufs=2) as qgpool,
            tc.tile_pool(name="ev", bufs=3) as evpool,
            tc.tile_pool(name="mmps", bufs=2, space="PSUM") as mmps,
            tc.tile_pool(name="dps", bufs=2, space="PSUM") as dps,
            tc.tile_pool(name="tps", bufs=2, space="PSUM") as tps,
        ):
            # ---- constants / weights resident in SBUF ----
            ident = cpool.tile([128, 128], dt.float32)
            make_identity(nc, ident[:])
            neghalf = cpool.tile([2, 128], dt.float16)
            nc.vector.memset(neghalf[:], -0.5)
            ones = cpool.tile([128, 1], dt.float32)
            nc.vector.memset(ones[:], 1.0)
            lossbuf = cpool.tile([128, NP16 * NSLOT], dt.float32)
            losst = cpool.tile([128, NP16], dt.float32)
            sqscratch = cpool.tile([128, CHUNK], dt.float32)

            wt = wpool.tile([128, 3 * 4 * DIM], dt.float32)  # conv lhsT tiles
            nc.sync.dma_start(out=wt[:].rearrange("p (t k f) -> p t k f", t=3, k=4),
                              in_=wf.rearrange("t k p f -> p t k f"))
            pwt = wpool.tile([128, 4 * DIM], dt.float32)
            nc.sync.dma_start(out=pwt[:].rearrange("p (k f) -> p k f", k=4),
                              in_=pwT.rearrange("k p f -> p k f"))
            b1t = wpool.tile([128, 4], dt.float32)
            nc.sync.dma_start(out=b1t[:], in_=b1.rearrange("k p -> p k"))
            pbt = wpool.tile([128, 4], dt.float32)
            nc.sync.dma_start(out=pbt[:], in_=pb.rearrange("k p -> p k"))

            def w_lhsT(tap, kt, m):
                base = (tap * 4 + kt) * DIM
                return wt[:, base + m * 128 : base + (m + 1) * 128]

            def pw_lhsT(kt, m):
                return pwt[:, kt * DIM + m * 128 : kt * DIM + (m + 1) * 128]

            for ci in range(NCHUNK):
                # ---- load x chunk (with halo) ----
                xc = xpool.tile([128, 4 * (CHUNK + 2)], dt.float32, tag="xc")
                nc.sync.dma_start(
                    out=xc[:].rearrange("p (k f) -> p k f", k=4),
                    in_=xp[:, :, ci * CHUNK : ci * CHUNK + CHUNK + 2].rearrange(
                        "k p f -> p k f"
                    ),
                )

                # ---- conv + pre (fused weights), fp32 exact ----
                h = hrpool.tile([128, 4 * CHUNK], dt.float32, tag="h")
                r = hrpool.tile([128, 4 * CHUNK], dt.float32, tag="r")
                for m in range(4):
                    for half in range(CHUNK // 512):
                        ps = mmps.tile([128, 512], dt.float32, tag="mm")
                        nmm = 0
                        for tap in range(3):
                            for kt in range(4):
                                base = kt * (CHUNK + 2) + half * 512 + tap
                                nc.tensor.matmul(
                                    ps[:],
                                    w_lhsT(tap, kt, m),
                                    xc[:, base : base + 512],
                                    start=(nmm == 0),
                                    stop=(nmm == 11),
                                )
                                nmm += 1
                        # evict + bias via ACT (bias is per-partition AP)
                        nc.scalar.activation(
                            h[:, m * CHUNK + half * 512 : m * CHUNK + half * 512 + 512],
                            ps[:],
                            AF.Identity,
                            bias=b1t[:, m : m + 1],
                        )
                # r = h
                for kt in range(4):
                    nc.vector.tensor_copy(
                        r[:, kt * CHUNK : (kt + 1) * CHUNK],
                        h[:, kt * CHUNK : (kt + 1) * CHUNK],
                    )

                # ---- VQ: 8 quantizers x 2 groups ----
                def vq_pair(ci, q, g, h, r):
                    p = q * 2 + g
                    # stream codebook (fp16 hi/lo) + c2 row
                    cbht = cbpool.tile([128, 2 * CSIZE], dt.float16, tag="cbh")
                    cblt = cbpool.tile([128, 2 * CSIZE], dt.float16, tag="cbl")
                    c2t = cbpool.tile([2, CSIZE], dt.float16, tag="c2")
                    nc.sync.dma_start(out=cbht[:].rearrange("p (k f) -> p k f", k=2), in_=cbh[p].rearrange("k p f -> p k f"))
                    nc.sync.dma_start(out=cblt[:].rearrange("p (k f) -> p k f", k=2), in_=cbl[p].rearrange("k p f -> p k f"))
                    nc.sync.dma_start(out=c2t[:], in_=c2hl[p])

                    # hi/lo split of residual for this group (fp16)
                    rh = hlpool.tile([128, 2 * CHUNK], dt.float16, tag="rh")
                    rl = hlpool.tile([128, 2 * CHUNK], dt.float16, tag="rl")
                    for kt in range(2):
                        rsl = r[:, (g * 2 + kt) * CHUNK : (g * 2 + kt + 1) * CHUNK]
                        nc.scalar.activation(
                            rh[:, kt * CHUNK : (kt + 1) * CHUNK], rsl, AF.Copy
                        )
                        nc.vector.tensor_sub(
                            rl[:, kt * CHUNK : (kt + 1) * CHUNK],
                            rsl,
                            rh[:, kt * CHUNK : (kt + 1) * CHUNK],
                        )

                    idxb = ipool.tile([128, PT * 8], dt.uint32, tag="idx")

                    def dist_tile(t):
                        dp = dps.tile([128, CSIZE], dt.float32, tag="dist")
                        # scores = dots - c2/2 (c2 via K=2 matmul with -0.5 lhsT)
                        for nh in range(2):
                            nsl = slice(nh * 512, nh * 512 + 512)
                            nc.tensor.matmul(
                                dp[:, nsl], neghalf[:], c2t[:, nsl],
                                start=True, stop=False,
                            )
                            for pas, (lt, ct) in enumerate(
                                ((rh, cbht), (rh, cblt), (rl, cbht))
                            ):
                                for kt in range(2):
                                    nc.tensor.matmul(
                                        dp[:, nsl],
                                        lt[:, kt * CHUNK + t * 128 : kt * CHUNK + t * 128 + 128],
                                        ct[:, kt * CSIZE + nh * 512 : kt * CSIZE + nh * 512 + 512],
                                        start=False,
                                        stop=(pas == 2 and kt == 1),
                                    )
                        ds = dpool.tile([128, CSIZE], dt.float32, tag="ds")
                        nc.scalar.activation(ds[:], dp[:], AF.Copy)
                        m8 = m8pool.tile([128, 8], dt.float32, tag="m8")
                        nc.vector.max(out=m8[:], in_=ds[:])
                        nc.vector.max_index(
                            out=idxb[:, t * 8 : t * 8 + 8], in_max=m8[:], in_values=ds[:]
                        )

                    for t in range(PT):
                        dist_tile(t)

                    # compact per-tile winning indices into contiguous layout
                    goff = ipool.tile([128, PT], dt.uint32, tag="goff")
                    nc.vector.tensor_copy(
                        goff[:], idxb.rearrange("p (t s) -> p t s", s=8)[:, :, 0]
                    )
                    # gather selected fp32 codebook rows (one row per partition
                    # per call -- hw consumes a single offset per partition)
                    quant = qgpool.tile([128, PT * DG], dt.float32, tag="quant")
                    for t in range(PT):
                        nc.gpsimd.indirect_dma_start(
                            out=quant[:, t * DG : (t + 1) * DG],
                            out_offset=None,
                            in_=cbr[:, :],
                            in_offset=bass.IndirectOffsetOnAxis(
                                ap=goff[:, t : t + 1], axis=0
                            ),
                            element_offset=p * CSIZE * DG,
                        )
                    # indices out, scrambled [g, q, ci, p, t]; host unscrambles
                    nc.sync.dma_start(
                        out=io[g, q, ci * CHUNK : (ci + 1) * CHUNK].rearrange(
                            "(p t) -> p t", t=PT
                        ),
                        in_=goff[:],
                    )

                    # residual update: r -= quant (transpose each tile via PE)
                    for t in range(PT):
                        for kt in range(2):
                            tp = tps.tile([128, 128], dt.float32, tag="tr")
                            nc.tensor.transpose(
                                out=tp[:],
                                in_=quant[:, t * DG + kt * 128 : t * DG + kt * 128 + 128],
                                identity=ident[:],
                            )
                            rsl = r[
                                :,
                                (g * 2 + kt) * CHUNK + t * 128 : (g * 2 + kt) * CHUNK
                                + t * 128
                                + 128,
                            ]
                            nc.vector.tensor_sub(rsl, rsl, tp[:])

                    # commit-loss partial: sum((r_new)^2) over this chunk
                    for kt in range(2):
                        slot = p * NSLOT + kt * NCHUNK + ci
                        nc.scalar.activation(
                            sqscratch[:],
                            r[:, (g * 2 + kt) * CHUNK : (g * 2 + kt + 1) * CHUNK],
                            AF.Square,
                            accum_out=lossbuf[:, slot : slot + 1],
                        )

                for q in range(NQ):
                    for g in range(GROUPS):
                        vq_pair(ci, q, g, h, r)

                # ---- qout = h - r (in place into h), then post linear ----
                for kt in range(4):
                    nc.vector.tensor_sub(
                        h[:, kt * CHUNK : (kt + 1) * CHUNK],
                        h[:, kt * CHUNK : (kt + 1) * CHUNK],
                        r[:, kt * CHUNK : (kt + 1) * CHUNK],
                    )
                for m in range(4):
                    for half in range(CHUNK // 512):
                        ps = mmps.tile([128, 512], dt.float32, tag="mm")
                        for kt in range(4):
                            nc.tensor.matmul(
                                ps[:],
                                pw_lhsT(kt, m),
                                h[:, kt * CHUNK + half * 512 : kt * CHUNK + half * 512 + 512],
                                start=(kt == 0),
                                stop=(kt == 3),
                            )
                        qe = evpool.tile([128, 512], dt.float32, tag="qe")
                        nc.scalar.activation(qe[:], ps[:], AF.Identity, bias=pbt[:, m : m + 1])
                        nc.sync.dma_start(
                            out=qo[m * 128 : (m + 1) * 128, ci * CHUNK + half * 512 : ci * CHUNK + half * 512 + 512],
                            in_=qe[:],
                        )

            # ---- finalize losses: [128, NP16*NSLOT] -> [1, NP16] ----
            for p in range(NP16):
                nc.vector.tensor_reduce(
                    losst[:, p : p + 1],
                    lossbuf[:, p * NSLOT : (p + 1) * NSLOT],
                    op=mybir.AluOpType.add,
                    axis=mybir.AxisListType.X,
                )
            lps = mmps.tile([1, NP16], dt.float32, tag="mm")
            nc.tensor.matmul(lps[:], ones[:], losst[:], start=True, stop=True)
            lss = m8pool.tile([1, NP16], dt.float32, tag="lss")
            nc.vector.tensor_copy(lss[:], lps[:])
            nc.sync.dma_start(out=lo[:, :], in_=lss[:])

    nc.compile()
    return nc


def _prep_inputs(x, conv_enc_w, conv_enc_b, pre_w, pre_b, codebooks, post_w, post_b):
    f64 = np.float64
    # fused conv+pre weights: W'_k = pre_w @ conv_w[:,:,k], stored transposed [din, dout]
    wf = np.empty((3, 4, 128, DIM), np.float32)
    for k in range(3):
        Wk = (pre_w.astype(f64) @ conv_enc_w[:, :, k].astype(f64)).T.astype(np.float32)
        wf[k] = Wk.reshape(4, 128, DIM)
    b1 = (pre_w.astype(f64) @ conv_enc_b.astype(f64) + pre_b.astype(f64)).astype(
        np.float32
    ).reshape(4, 128)

    cbh = np.empty((NP16, 2, 128, CSIZE), np.float16)
    cbl = np.empty((NP16, 2, 128, CSIZE), np.float16)
    c2hl = np.empty((NP16, 2, CSIZE), np.float16)
    cbr = np.empty((NP16 * CSIZE, DG), np.float32)
    for q in range(NQ):
        for g in range(GROUPS):
            p = q * 2 + g
            cb = codebooks[q, g]                     # [C, DG] fp32
            cbT = cb.T.astype(np.float32)            # [DG, C]
            hi = cbT.astype(np.float16)
            lor = (cbT - hi.astype(np.float32)).astype(np.float16)
            cbh[p] = hi.reshape(2, 128, CSIZE)
            cbl[p] = lor.reshape(2, 128, CSIZE)
            c2 = np.sum(cb * cb, axis=-1, dtype=np.float32)   # [C]
            c2h = c2.astype(np.float16)
            c2hl[p, 0] = c2h
            c2hl[p, 1] = (c2 - c2h.astype(np.float32)).astype(np.float16)
            cbr[p * CSIZE : (p + 1) * CSIZE] = cb

    pwT = post_w.T.astype(np.float32).copy().reshape(4, 128, DIM)
    pb = post_b.astype(np.float32).reshape(4, 128)

    xp = np.zeros((B, 4, 128, L + 2), np.float32)
    xp[:, :, :, 1 : L + 1] = np.asarray(x, np.float32).reshape(B, 4, 128, L)

    shared = dict(wf=wf, b1=b1, cbh=cbh, cbl=cbl, c2hl=c2hl, cbr=cbr, pwT=pwT, pb=pb)
    return xp, shared


def kernel(x, conv_enc_w, conv_enc_b, pre_w, pre_b, codebooks, post_w, post_b,
           conv_dec_w=None, conv_dec_b=None):
    from concourse.bass_utils import run_bass_kernel_spmd

    x = np.asarray(x, np.float32)
    xp, shared = _prep_inputs(
        np.asarray(x, np.float32), np.asarray(conv_enc_w, np.float32),
        np.asarray(conv_enc_b, np.float32), np.asarray(pre_w, np.float32),
        np.asarray(pre_b, np.float32), np.asarray(codebooks, np.float32),
        np.asarray(post_w, np.float32), np.asarray(post_b, np.float32),
    )
    import os
    nc = _build()
    in_maps = [dict(xp=xp[c], **shared) for c in range(B)]
    kw = {}
    if os.environ.get("GRVQ_TRACE", "") == "1":
        kw = dict(trace=True, tmpdir=os.environ.get("GRVQ_TRACE_DIR") or None)
    br = run_bass_kernel_spmd(nc, in_maps, list(range(B)), **kw)
    globals()["LAST_RESULT"] = br
    res = br.results

    q = np.stack([res[c]["qo"] for c in range(B)])                    # [B, D, L]
    idx = np.stack([res[c]["io"] for c in range(B)])                  # [B, G, Q, L']
    # unscramble l' = (ci, p, t) -> l = ci*CHUNK + t*128 + p
    idx = idx.reshape(B, GROUPS, NQ, NCHUNK, 128, PT).transpose(0, 1, 2, 3, 5, 4)
    idx = idx.reshape(B, GROUPS, NQ, L)
    indices = idx.transpose(1, 0, 3, 2).astype(np.int32)              # [G, B, L, Q]
    losum = np.stack([res[c]["lo"][0] for c in range(B)]).sum(0)      # [NP16]
    losses = (losum.reshape(NQ, GROUPS).T / np.float32(B * L * DG)).astype(np.float32)
    return q, indices, losses
